# revision 2
# baseline (speedup 1.0000x reference)
"""Trainium2 Bass kernel for nn_ObjectRaysampler.

Full (unsharded) inputs -> full outputs. Rays are sharded across 8 NeuronCores
(data-parallel); the tiny object transforms are replicated.

Per 128-ray tile on device:
  - world->object transform of ray origins/directions (row-vector convention),
    slab ray-AABB test against the unit box, entry/exit t.
  - world-space sample depths are z_w = c * z_o with c = ||d|| / ||d @ (R S)||,
    so each object's 32 samples form an ascending arithmetic progression:
    z(s) = A + B*(s/31), A = c*t_in, B = c*(tmax-t_in); misses get exactly 1e10.
  - sort keys: the f32 depth with its low 5 mantissa bits replaced by
    (node_id+1) for hits and 0 for misses/base entries. Keys stay positive
    floats, so min/max comparisons give the same order as the reference's
    stable argsort (ties only occur between interchangeable entries), and
    node id / hit mask are recovered from the sorted keys by bit masking.
    The <=31-ulp key perturbation is ~4e-6 relative on output lengths.
  - the per-ray 576 values are 17 presorted runs (16 arithmetic progressions
    + the presorted base lengths), merged with a bitonic-merge network
    (40 stages, 95 min/max ops) instead of a full sort; the final merge's
    upper half is pruned to the 64 slots that can reach the output.
  - object-space sample points/dirs are computed on GpSimd and streamed out.
"""

import contextlib
import numpy as np

from concourse import bacc, tile, mybir
from concourse.alu_op_type import AluOpType
from concourse.bass_utils import run_bass_kernel_spmd

N_RAYS = 32768
M = 16          # objects
S = 32          # samples per object
B = 64          # base samples
K = B + M * S   # 576
PADK = 1024
MISS = 1e10
PAD_SENTINEL = 1e30
N_CORES = 8
P = 128                       # rays per tile (partition dim)
CORE_RAYS = N_RAYS // N_CORES # 4096
TILES = CORE_RAYS // P        # 32
F32 = mybir.dt.float32
I32 = mybir.dt.int32
U8 = mybir.dt.uint8


# ---------------------------------------------------------------- sort network

def _flip(L, lo, hi):
    nblk = (hi - lo) // (2 * L)
    q = 2 * L
    return [
        ("min", lo, nblk, q, slice(0, L), slice(0, L), slice(2 * L - 1, L - 1, -1)),
        ("max", lo, nblk, q, slice(L, 2 * L), slice(L, 2 * L), slice(L - 1, None, -1)),
    ]


def _plain(d, lo, hi, minonly=False):
    nblk = (hi - lo) // (2 * d)
    q = 2 * d
    ops = [("min", lo, nblk, q, slice(0, d), slice(0, d), slice(d, 2 * d))]
    if not minonly:
        ops.append(("max", lo, nblk, q, slice(d, 2 * d), slice(0, d), slice(d, 2 * d)))
    return ops


def build_stages():
    stages = []
    for L in (32, 64, 128, 256):            # merge 32-runs -> one 512-run
        stages.append(_flip(L, 0, 512))
        d = L // 2
        while d >= 1:
            stages.append(_plain(d, 0, 512))
            d //= 2
    stages.append(_flip(512, 0, 1024))      # final merge with base+pad
    for d in (256, 128, 64, 32, 16, 8, 4, 2, 1):
        ops = _plain(d, 0, 512)
        if d >= 64:
            ops += _plain(d, 512, 512 + 2 * d, minonly=True)
        else:
            ops += _plain(d, 512, 576)
        stages.append(ops)
    return stages


_ALU = {"min": AluOpType.min, "max": AluOpType.max}


def _emit_stage_ops(eng, dst, src, stage):
    for (alu, off, nblk, q, o_sl, a_sl, b_sl) in stage:
        ov = dst[:, off:off + nblk * q].rearrange("p (b q) -> p b q", b=nblk)[:, :, o_sl]
        av = src[:, off:off + nblk * q].rearrange("p (b q) -> p b q", b=nblk)[:, :, a_sl]
        bv = src[:, off:off + nblk * q].rearrange("p (b q) -> p b q", b=nblk)[:, :, b_sl]
        eng.tensor_tensor(ov, av, bv, _ALU[alu])


# ---------------------------------------------------------------- device kernel

def object_raysampler_kernel(tc, outs, ins, n_rays=CORE_RAYS):
    nc = tc.nc
    n_tiles = n_rays // P
    stages = build_stages()

    org_d, dir_d, len_d = ins["origins"], ins["directions"], ins["lengths"]
    traf_d, rots_d, sdiag_d = ins["traf"], ins["rots"], ins["sdiag"]
    iota_d, nodef_d = ins["iota31"], ins["nodef"]
    slen_d, snode_d, smask_d = outs["slen"], outs["snode"], outs["smask"]
    pts_d, dirs_d = outs["pts"], outs["dirso"]

    with contextlib.ExitStack() as ctx:
        cpool = ctx.enter_context(tc.tile_pool(name="const", bufs=1))
        pool = ctx.enter_context(tc.tile_pool(name="work", bufs=2))

        # ---- constants / transforms (once) ----
        traf_t = cpool.tile([1, 192], F32)
        nc.sync.dma_start(traf_t[:], traf_d[:])
        rots_t = cpool.tile([1, 192], F32)
        nc.sync.dma_start(rots_t[:], rots_d[:])
        sdiag_t = cpool.tile([1, 192], F32)
        nc.sync.dma_start(sdiag_t[:], sdiag_d[:])
        iota_r = cpool.tile([1, 512], F32)
        nc.sync.dma_start(iota_r[:], iota_d[:])
        nodef_r = cpool.tile([1, 16], F32)
        nc.sync.dma_start(nodef_r[:], nodef_d[:])

        pt1 = cpool.tile([1, 192], F32)   # trafos @ scales, rows (i, (j,m))
        nc.vector.tensor_mul(pt1[:], traf_t[:], sdiag_t[:])
        dt1 = cpool.tile([1, 192], F32)   # rots @ scales
        nc.vector.tensor_mul(dt1[:], rots_t[:], sdiag_t[:])

        ptb = []
        for i in range(4):
            t = cpool.tile([P, 48], F32, tag=f"ptb{i}")
            nc.gpsimd.partition_broadcast(t[:], pt1[:, i * 48:(i + 1) * 48])
            ptb.append(t)
        dtb = []
        for i in range(3):
            t = cpool.tile([P, 48], F32, tag=f"dtb{i}")
            nc.gpsimd.partition_broadcast(t[:], dt1[:, i * 48:(i + 1) * 48])
            dtb.append(t)
        iota_b = cpool.tile([P, 512], F32)
        nc.gpsimd.partition_broadcast(iota_b[:], iota_r[:])
        nodef_b = cpool.tile([P, 16], F32)
        nc.gpsimd.partition_broadcast(nodef_b[:], nodef_r[:])
        c1e10 = cpool.tile([P, 16], F32)
        nc.vector.memset(c1e10[:], MISS)
        czero = cpool.tile([P, 16], F32)
        nc.vector.memset(czero[:], 0.0)

        iota_v = iota_b[:].rearrange("p (m s) -> p m s", m=M)

        TT = nc.vector.tensor_tensor
        TS = nc.vector.tensor_scalar
        STT = nc.vector.scalar_tensor_tensor
        gTT = nc.gpsimd.tensor_tensor

        for t in range(n_tiles):
            r0 = t * P
            org = pool.tile([P, 3], F32)
            nc.sync.dma_start(org[:], org_d[r0:r0 + P, :])
            dirw = pool.tile([P, 3], F32)
            nc.sync.dma_start(dirw[:], dir_d[r0:r0 + P, :])

            # ||d||
            d2 = pool.tile([P, 3], F32)
            TT(d2[:], dirw[:], dirw[:], AluOpType.mult)
            nd = pool.tile([P, 1], F32)
            TT(nd[:], d2[:, 0:1], d2[:, 1:2], AluOpType.add)
            TT(nd[:], nd[:], d2[:, 2:3], AluOpType.add)
            nrm = pool.tile([P, 1], F32)
            nc.scalar.activation(nrm[:], nd[:], mybir.ActivationFunctionType.Sqrt)

            # origins/dirs in object frames: layout (j, m) 48-wide
            oo = pool.tile([P, 48], F32)
            TS(oo[:], ptb[0][:], org[:, 0:1], None, AluOpType.mult)
            STT(oo[:], ptb[1][:], org[:, 1:2], oo[:], AluOpType.mult, AluOpType.add)
            STT(oo[:], ptb[2][:], org[:, 2:3], oo[:], AluOpType.mult, AluOpType.add)
            TT(oo[:], oo[:], ptb[3][:], AluOpType.add)

            uu = pool.tile([P, 48], F32)
            TS(uu[:], dtb[0][:], dirw[:, 0:1], None, AluOpType.mult)
            STT(uu[:], dtb[1][:], dirw[:, 1:2], uu[:], AluOpType.mult, AluOpType.add)
            STT(uu[:], dtb[2][:], dirw[:, 2:3], uu[:], AluOpType.mult, AluOpType.add)

            u2 = pool.tile([P, 48], F32)
            TT(u2[:], uu[:], uu[:], AluOpType.mult)
            n2 = pool.tile([P, 16], F32)
            TT(n2[:], u2[:, 0:16], u2[:, 16:32], AluOpType.add)
            TT(n2[:], n2[:], u2[:, 32:48], AluOpType.add)
            no = pool.tile([P, 16], F32)
            nc.scalar.activation(no[:], n2[:], mybir.ActivationFunctionType.Sqrt)
            rno = pool.tile([P, 16], F32)
            nc.vector.reciprocal(rno[:], no[:])

            cc = pool.tile([P, 16], F32)
            TS(cc[:], rno[:], nrm[:, 0:1], None, AluOpType.mult)

            un = pool.tile([P, 48], F32)
            TT(un[:].rearrange("p (j m) -> p j m", j=3), uu[:].rearrange("p (j m) -> p j m", j=3),
               rno[:].unsqueeze(1).broadcast_to([P, 3, 16]), AluOpType.mult)

            inv = pool.tile([P, 48], F32)
            nc.vector.reciprocal(inv[:], un[:])

            w0 = pool.tile([P, 48], F32)   # -oo - 1
            TS(w0[:], oo[:], -1.0, -1.0, AluOpType.mult, AluOpType.add)
            t0 = pool.tile([P, 48], F32)
            TT(t0[:], w0[:], inv[:], AluOpType.mult)
            w1 = pool.tile([P, 48], F32)   # -oo + 1
            TS(w1[:], oo[:], -1.0, 1.0, AluOpType.mult, AluOpType.add)
            t1 = pool.tile([P, 48], F32)
            TT(t1[:], w1[:], inv[:], AluOpType.mult)

            lo = pool.tile([P, 48], F32)
            TT(lo[:], t0[:], t1[:], AluOpType.min)
            hi = pool.tile([P, 48], F32)
            TT(hi[:], t0[:], t1[:], AluOpType.max)

            tmn = pool.tile([P, 16], F32)
            TT(tmn[:], lo[:, 0:16], lo[:, 16:32], AluOpType.max)
            TT(tmn[:], tmn[:], lo[:, 32:48], AluOpType.max)
            tmx = pool.tile([P, 16], F32)
            TT(tmx[:], hi[:, 0:16], hi[:, 16:32], AluOpType.min)
            TT(tmx[:], tmx[:], hi[:, 32:48], AluOpType.min)

            hit = pool.tile([P, 16], F32)
            TT(hit[:], tmx[:], tmn[:], AluOpType.is_gt)
            h2 = pool.tile([P, 16], F32)
            TS(h2[:], tmx[:], 0.0, None, AluOpType.is_gt)
            TT(hit[:], hit[:], h2[:], AluOpType.mult)

            tin = pool.tile([P, 16], F32)
            TS(tin[:], tmn[:], 0.0, None, AluOpType.max)
            wdt = pool.tile([P, 16], F32)   # tmax - t_in
            TT(wdt[:], tmx[:], tin[:], AluOpType.subtract)

            a0 = pool.tile([P, 16], F32)
            TT(a0[:], cc[:], tin[:], AluOpType.mult)
            b0 = pool.tile([P, 16], F32)
            TT(b0[:], cc[:], wdt[:], AluOpType.mult)

            am = pool.tile([P, 16], F32)
            nc.vector.tensor_copy(am[:], c1e10[:])
            nc.vector.copy_predicated(am[:], hit[:].bitcast(I32), a0[:])
            bm = pool.tile([P, 16], F32)
            nc.vector.tensor_copy(bm[:], czero[:])
            nc.vector.copy_predicated(bm[:], hit[:].bitcast(I32), b0[:])

            nbf = pool.tile([P, 16], F32)
            TT(nbf[:], hit[:], nodef_b[:], AluOpType.mult)
            nbi = pool.tile([P, 16], I32)
            nc.vector.tensor_copy(nbi[:], nbf[:])

            # ---- keys ----
            ka = pool.tile([P, PADK], F32)
            kb = pool.tile([P, PADK], F32)
            kav = ka[:, 0:512].rearrange("p (m s) -> p m s", m=M)
            TT(kav, bm[:].unsqueeze(2).broadcast_to([P, M, S]), iota_v, AluOpType.mult)
            TT(kav, kav, am[:].unsqueeze(2).broadcast_to([P, M, S]), AluOpType.add)
            kai = ka[:].bitcast(I32)
            TS(kai[:, 0:512], kai[:, 0:512], -32, None, AluOpType.bitwise_and)
            TT(kai[:, 0:512].rearrange("p (m s) -> p m s", m=M),
               kai[:, 0:512].rearrange("p (m s) -> p m s", m=M),
               nbi[:].unsqueeze(2).broadcast_to([P, M, S]), AluOpType.bitwise_or)
            nc.sync.dma_start(ka[:, 512:576], len_d[r0:r0 + P, :])
            TS(kai[:, 512:576], kai[:, 512:576], -32, None, AluOpType.bitwise_and)
            nc.gpsimd.memset(ka[:, 576:PADK], PAD_SENTINEL)

            # ---- merge network (ping-pong ka <-> kb) ----
            src, dst = ka, kb
            for stage in stages:
                _emit_stage_ops(nc.vector, dst[:], src[:], stage)
                src, dst = dst, src
            skey = src  # == ka (40 stages)

            # ---- extraction ----
            it5 = pool.tile([P, K], I32)
            TS(it5[:], skey[:].bitcast(I32)[:, 0:K], 31, None, AluOpType.bitwise_and)
            ft5 = pool.tile([P, K], F32)
            nc.vector.tensor_copy(ft5[:], it5[:])
            node_t = pool.tile([P, K], I32)
            TS(node_t[:], ft5[:], 1.0, None, AluOpType.subtract)
            mask_t = pool.tile([P, K], U8)
            TS(mask_t[:], ft5[:], 0.0, None, AluOpType.is_gt)

            nc.sync.dma_start(slen_d[r0:r0 + P, :], skey[:, 0:K])
            nc.sync.dma_start(snode_d[r0:r0 + P, :], node_t[:])
            nc.sync.dma_start(smask_d[r0:r0 + P, :], mask_t[:])

            # ---- sample points / dirs (GpSimd) ----
            zo = pool.tile([P, 512], F32)
            zov = zo[:].rearrange("p (m s) -> p m s", m=M)
            gTT(zov, wdt[:].unsqueeze(2).broadcast_to([P, M, S]), iota_v, AluOpType.mult)
            gTT(zov, zov, tin[:].unsqueeze(2).broadcast_to([P, M, S]), AluOpType.add)

            hit_b = hit[:].unsqueeze(2).broadcast_to([P, M, S])
            pts_t = pool.tile([P, M * S * 3], F32)
            dirs_t = pool.tile([P, M * S * 3], F32)
            ptsv = pts_t[:].rearrange("p (m s j) -> p m s j", m=M, s=S)
            dirsv = dirs_t[:].rearrange("p (m s j) -> p m s j", m=M, s=S)
            tmp = pool.tile([P, 512], F32)
            tmpv = tmp[:].rearrange("p (m s) -> p m s", m=M)
            for j in range(3):
                unj = un[:, j * 16:(j + 1) * 16].unsqueeze(2).broadcast_to([P, M, S])
                ooj = oo[:, j * 16:(j + 1) * 16].unsqueeze(2).broadcast_to([P, M, S])
                gTT(tmpv, unj, zov, AluOpType.mult)
                gTT(tmpv, tmpv, ooj, AluOpType.add)
                gTT(ptsv[:, :, :, j], tmpv, hit_b, AluOpType.mult)
                gTT(dirsv[:, :, :, j], unj, hit_b, AluOpType.mult)

            nc.sync.dma_start(pts_d[:, r0:r0 + P, :].transpose([1, 0, 2]),
                              pts_t[:].rearrange("p (m f) -> p m f", m=M))
            nc.sync.dma_start(dirs_d[:, r0:r0 + P, :].transpose([1, 0, 2]),
                              dirs_t[:].rearrange("p (m f) -> p m f", m=M))


# ---------------------------------------------------------------- host wrapper

def _make_const_inputs(trafos_w2o, rots_w2o, scales_w2o):
    # layout rows (i, (j, m)) flattened to [1, 192]
    traf = np.ascontiguousarray(trafos_w2o[:, 0:4, 0:3].transpose(1, 2, 0)).reshape(1, 192)
    rots = np.ascontiguousarray(rots_w2o[:, 0:4, 0:3].transpose(1, 2, 0)).reshape(1, 192)
    sd = np.stack([scales_w2o[:, 0, 0], scales_w2o[:, 1, 1], scales_w2o[:, 2, 2]], axis=0)  # [3, M] = (j, m)
    sdiag = np.tile(sd.reshape(1, 48), (1, 4)).reshape(4, 48)[:, :]  # rows i=0..3 identical
    sdiag = sdiag.reshape(1, 192).astype(np.float32)
    lin = np.linspace(0.0, 1.0, S, dtype=np.float32)
    iota31 = np.tile(lin, M).reshape(1, 512).astype(np.float32)
    nodef = (np.arange(M, dtype=np.float32) + 1.0).reshape(1, 16)
    return (traf.astype(np.float32), rots.astype(np.float32), sdiag, iota31, nodef)


_COMPILED = {}


def _get_compiled():
    if "nc" in _COMPILED:
        return _COMPILED["nc"]
    nc = bacc.Bacc("TRN2", target_bir_lowering=False, debug=False,
                   num_devices=N_CORES)
    ins = {
        "origins": nc.dram_tensor("origins", [CORE_RAYS, 3], F32, kind="ExternalInput").ap(),
        "directions": nc.dram_tensor("directions", [CORE_RAYS, 3], F32, kind="ExternalInput").ap(),
        "lengths": nc.dram_tensor("lengths", [CORE_RAYS, B], F32, kind="ExternalInput").ap(),
        "traf": nc.dram_tensor("traf", [1, 192], F32, kind="ExternalInput").ap(),
        "rots": nc.dram_tensor("rots", [1, 192], F32, kind="ExternalInput").ap(),
        "sdiag": nc.dram_tensor("sdiag", [1, 192], F32, kind="ExternalInput").ap(),
        "iota31": nc.dram_tensor("iota31", [1, 512], F32, kind="ExternalInput").ap(),
        "nodef": nc.dram_tensor("nodef", [1, 16], F32, kind="ExternalInput").ap(),
    }
    outs = {
        "slen": nc.dram_tensor("slen", [CORE_RAYS, K], F32, kind="ExternalOutput").ap(),
        "snode": nc.dram_tensor("snode", [CORE_RAYS, K], I32, kind="ExternalOutput").ap(),
        "smask": nc.dram_tensor("smask", [CORE_RAYS, K], U8, kind="ExternalOutput").ap(),
        "pts": nc.dram_tensor("pts", [M, CORE_RAYS, S * 3], F32, kind="ExternalOutput").ap(),
        "dirso": nc.dram_tensor("dirso", [M, CORE_RAYS, S * 3], F32, kind="ExternalOutput").ap(),
    }
    with tile.TileContext(nc) as tc:
        object_raysampler_kernel(tc, outs, ins)
    nc.compile()
    _COMPILED["nc"] = nc
    return nc


def kernel(origins, directions, lengths, trafos_w2o, rots_w2o, scales_w2o,
           _trace=False, _trace_kwargs=None):
    origins = np.asarray(origins, dtype=np.float32)
    directions = np.asarray(directions, dtype=np.float32)
    lengths = np.asarray(lengths, dtype=np.float32)
    traf, rots, sdiag, iota31, nodef = _make_const_inputs(
        np.asarray(trafos_w2o, np.float32), np.asarray(rots_w2o, np.float32),
        np.asarray(scales_w2o, np.float32))

    nc = _get_compiled()
    in_maps = []
    for c in range(N_CORES):
        r0 = c * CORE_RAYS
        in_maps.append({
            "origins": origins[r0:r0 + CORE_RAYS],
            "directions": directions[r0:r0 + CORE_RAYS],
            "lengths": lengths[r0:r0 + CORE_RAYS],
            "traf": traf, "rots": rots, "sdiag": sdiag,
            "iota31": iota31, "nodef": nodef,
        })
    kwargs = {}
    if _trace:
        kwargs = dict(trace=True, **(_trace_kwargs or {}))
    res = run_bass_kernel_spmd(nc, in_maps, list(range(N_CORES)), **kwargs)
    results = res.results

    slen = np.concatenate([results[c]["slen"] for c in range(N_CORES)], axis=0)
    snode = np.concatenate([results[c]["snode"] for c in range(N_CORES)], axis=0)
    smask = np.concatenate([results[c]["smask"] for c in range(N_CORES)], axis=0)
    pts = np.concatenate([results[c]["pts"] for c in range(N_CORES)], axis=1)
    dirso = np.concatenate([results[c]["dirso"] for c in range(N_CORES)], axis=1)
    pts_flat = pts.reshape(-1, 3)
    dirs_flat = dirso.reshape(-1, 3)
    out = (slen, snode.astype(np.int32), smask.astype(bool), pts_flat, dirs_flat)
    if _trace:
        return out, res
    return out
